# revision 8
# baseline (speedup 1.0000x reference)
"""RBF-kernel SVM decision function on 8 TRN2 NeuronCores.

out[i] = sum_j alphas[j] * exp(-GAMMA * ||x[i] - supports[j]||^2)

Strategy (data-parallel over x rows, supports/alphas replicated):
  exponent e_ij = -g|x_i|^2 + (2g x_i . s_j) + (ln|a_j| - g|s_j|^2)
    - 2g x_i.s_j  : bf16 matmul, x-side scaled by 1/32 (exact pow2), s-side raw
    - j-term      : folded into the matmul as 2 extra contraction rows (hi/lo
                    bf16 split for ~fp24 accuracy), x-side rows = 1.0
    - i-term      : fp32 per-partition bias of the ACTIVATE(Exp)
  out_i = sum_{j: a_j>0} exp(e_ij) - sum_{j: a_j<0} exp(e_ij)
    - supports host-sorted so positive-alpha group comes first
    - ACTIVATE(Exp, accum_out=...) reduces along the free dim in the same
      pass as the exp; DVE reduces the per-chunk partials and subtracts.
"""

import sys

for p in ("/opt/trn_rl_repo",):
    if p not in sys.path:
        sys.path.insert(0, p)

import numpy as np
import ml_dtypes

import concourse.bass as bass
import concourse.tile as tile
from concourse import bacc, mybir
from concourse.bass_utils import run_bass_kernel_spmd

N_CORES = 8
N = 16384
M = 8192
F = 64
GAMMA = 1.0 / F
N_LOC = N // N_CORES        # 2048 queries per core
N_TILES = N_LOC // 128      # 16 i-tiles of 128 queries
K_AUG = F + 2               # 66 contraction rows
W = 2048                    # j-window: 4 PSUM banks
NW = M // W                 # 4 windows per j sweep
MM_N = 512                  # matmul moving free dim (1 PSUM bank)

BF16 = mybir.dt.bfloat16
F32 = mybir.dt.float32
bf16 = ml_dtypes.bfloat16

_compiled_cache = {}


def _build(b):
    nc = bacc.Bacc(
        "TRN2",
        target_bir_lowering=False,
        debug=False,
        enable_asserts=False,
        num_devices=N_CORES,
    )
    xaugT_d = nc.dram_tensor("xaugT", [K_AUG, N_LOC], BF16, kind="ExternalInput")
    saug_d = nc.dram_tensor("saug", [K_AUG, M], BF16, kind="ExternalInput")
    cbias_d = nc.dram_tensor("cbias", [128, N_TILES], F32, kind="ExternalInput")
    out_d = nc.dram_tensor("out", [128, N_TILES], F32, kind="ExternalOutput")

    # Window w is pure-positive (w < w_mix), pure-negative (w > w_mix), or the
    # single sign-mixed window w_mix, which is reduced on the DVE against a
    # +/-1 tile instead of splitting the ACTIVATE.
    import os as _os

    use_dve_mix = bool(b % W) and not _os.environ.get("NO_DVE_MIX")
    w_mix = b // W if use_dve_mix else -1
    n_pos = sum(1 for w in range(NW) if w * W < b) + (1 if use_dve_mix else 0)
    n_neg = sum(1 for w in range(NW) if (w + 1) * W > b) + (1 if use_dve_mix else 0)

    with tile.TileContext(nc) as tc:
        with (
            tc.tile_pool(name="const", bufs=1) as cpool,
            tc.tile_pool(name="acc", bufs=3) as apool,
            tc.tile_pool(name="psum", bufs=2, space="PSUM") as ppool,
        ):
            # Dummy exp() on a zeroed tile: first in the ACT engine's program,
            # so the ~2.7us exp table load overlaps the input DMAs instead of
            # stalling the first real ACTIVATE.
            warm_act = cpool.tile([128, 1], F32)
            nc.gpsimd.memset(warm_act[:], 0.0)
            nc.scalar.activation(
                warm_act[:], warm_act[:], mybir.ActivationFunctionType.Exp
            )

            # +/-1 signs for the mixed window (built by memset, no DMA).
            if w_mix >= 0:
                sign_sb = cpool.tile([128, W], F32)
                split = b - w_mix * W
                nc.gpsimd.memset(sign_sb[:, :split], 1.0)
                nc.gpsimd.memset(sign_sb[:, split:], -1.0)

            # DMA order: the first j-window of saug and the first x-tile gate
            # the start of compute — issue those first, split across the sync
            # (HWDGE) and gpsimd (SWDGE) engines so issue doesn't serialize.
            saug_sb = cpool.tile([K_AUG, M], BF16)
            nc.sync.dma_start(saug_sb[:, 0:W], saug_d.ap()[:, 0:W])
            xaugT_sb = cpool.tile([K_AUG, N_LOC], BF16)
            nc.gpsimd.dma_start(xaugT_sb[:, 0:128], xaugT_d.ap()[:, 0:128])
            cbias_sb = cpool.tile([128, N_TILES], F32)
            nc.gpsimd.dma_start(cbias_sb[:], cbias_d.ap()[:])
            for w in range(1, NW):
                nc.sync.dma_start(
                    saug_sb[:, w * W : (w + 1) * W],
                    saug_d.ap()[:, w * W : (w + 1) * W],
                )
            nc.gpsimd.dma_start(xaugT_sb[:, 128:], xaugT_d.ap()[:, 128:])
            outT_sb = cpool.tile([128, N_TILES], F32)

            for t in range(N_TILES):
                accP = apool.tile([128, max(n_pos, 1)], F32, tag="accP")
                accN = apool.tile([128, max(n_neg, 1)], F32, tag="accN")
                iP = iN = 0
                for w in range(NW):
                    ps_tile = ppool.tile([128, W], F32, tag="E")
                    for c in range(W // MM_N):
                        nc.tensor.matmul(
                            ps_tile[:, c * MM_N : (c + 1) * MM_N],
                            xaugT_sb[:, t * 128 : (t + 1) * 128],
                            saug_sb[:, w * W + c * MM_N : w * W + (c + 1) * MM_N],
                            start=True,
                            stop=True,
                        )
                    if w == w_mix:
                        # exp without accumulation, then signed reduce on DVE.
                        nc.scalar.activation(
                            ps_tile[:],
                            ps_tile[:],
                            mybir.ActivationFunctionType.Exp,
                            bias=cbias_sb[:, t : t + 1],
                        )
                        acc_col = accP[:, iP : iP + 1]
                        iP += 1
                        nc.vector.tensor_tensor_reduce(
                            ps_tile[:],
                            ps_tile[:],
                            sign_sb[:],
                            1.0,
                            0.0,
                            mybir.AluOpType.mult,
                            mybir.AluOpType.add,
                            acc_col,
                        )
                    else:
                        lo, hi = w * W, (w + 1) * W
                        if b <= lo:
                            pieces = [(lo, hi, False)]
                        elif b >= hi:
                            pieces = [(lo, hi, True)]
                        else:
                            pieces = [(lo, b, True), (b, hi, False)]
                        for plo, phi, pos in pieces:
                            if pos:
                                acc_col = accP[:, iP : iP + 1]
                                iP += 1
                            else:
                                acc_col = accN[:, iN : iN + 1]
                                iN += 1
                            nc.scalar.activation(
                                ps_tile[:, plo - lo : phi - lo],
                                ps_tile[:, plo - lo : phi - lo],
                                mybir.ActivationFunctionType.Exp,
                                bias=cbias_sb[:, t : t + 1],
                                accum_out=acc_col,
                            )
                sumP = apool.tile([128, 1], F32, tag="sumP")
                nc.vector.reduce_sum(sumP[:], accP[:, :iP], axis=mybir.AxisListType.X)
                sumN = apool.tile([128, 1], F32, tag="sumN")
                nc.vector.reduce_sum(sumN[:], accN[:, :iN], axis=mybir.AxisListType.X)
                nc.vector.tensor_sub(outT_sb[:, t : t + 1], sumP[:], sumN[:])

            nc.sync.dma_start(out_d.ap()[:], outT_sb[:])

    nc.compile()
    return nc


def _prepare(x, supports, alphas):
    x = np.asarray(x, dtype=np.float32)
    supports = np.asarray(supports, dtype=np.float32)
    alphas = np.asarray(alphas, dtype=np.float32)

    a64 = alphas.astype(np.float64)
    s64 = supports.astype(np.float64)
    jterm = -GAMMA * (s64 * s64).sum(axis=1) + np.log(
        np.maximum(np.abs(a64), 1e-300)
    )

    pos = a64 > 0
    perm = np.concatenate([np.nonzero(pos)[0], np.nonzero(~pos)[0]])
    b = int(pos.sum())

    jt = jterm[perm]
    hi = jt.astype(bf16)
    lo = (jt - hi.astype(np.float64)).astype(bf16)

    saug = np.empty((K_AUG, M), dtype=bf16)
    saug[:F] = supports[perm].T.astype(bf16)
    saug[F] = hi
    saug[F + 1] = lo

    xaugT = np.ones((K_AUG, N), dtype=bf16)
    xaugT[:F] = (x.T / 32.0).astype(bf16)

    cbias = (-GAMMA * (x.astype(np.float64) ** 2).sum(axis=1)).astype(np.float32)

    in_maps = []
    for c in range(N_CORES):
        sl = slice(c * N_LOC, (c + 1) * N_LOC)
        in_maps.append(
            {
                "xaugT": np.ascontiguousarray(xaugT[:, sl]),
                "saug": saug,
                "cbias": np.ascontiguousarray(
                    cbias[sl].reshape(N_TILES, 128).T
                ),
            }
        )
    return b, in_maps


def _run(x, supports, alphas, trace=False, **run_kwargs):
    b, in_maps = _prepare(x, supports, alphas)
    if b not in _compiled_cache:
        _compiled_cache[b] = _build(b)
    nc = _compiled_cache[b]
    res = run_bass_kernel_spmd(
        nc, in_maps, core_ids=list(range(N_CORES)), trace=trace, **run_kwargs
    )
    outs = [r["out"].T.reshape(-1) for r in res.results]
    return np.concatenate(outs).astype(np.float32), res


def kernel(x, supports, alphas):
    out, _ = _run(x, supports, alphas, trace=False)
    return out
